# revision 9
# baseline (speedup 1.0000x reference)
"""Trainium2 Bass kernel for the hierarchical softmax loss problem.

Tree is the fixed balanced tree from the reference: K=10, D=4, L=10000,
node order leaves..root. Data-parallel over batch: 8 cores x 256 rows.

Device work per core (2 tiles of 128 rows):
  - exp(x) over [128,10000] on ACT (accum_out gives the softmax denominator)
  - grouped reduce exp -> p2 [128,100] (level-2 node sums) on DVE
  - greedy argmax descent root->leaf using p2/p3 on-chip plus one
    indirect-DMA gather of the 100 leaf logits under the chosen level-2 node
  - win_u = sum of path probs * 2^-weights via two small dot products
    (host-precomputed weight vectors; the target's 10-leaf block is
    fetched with a second indirect-DMA gather)
Host: final tiny reductions (mean of 2048 logs, preds concat, total_dist).
"""

import sys

if "/opt/trn_rl_repo" not in sys.path:
    sys.path.insert(0, "/opt/trn_rl_repo")

import numpy as np

import concourse.bass as bass
import concourse.tile as tile
from concourse import mybir
from concourse.bass_utils import run_bass_kernel_spmd
from concourse.vector_clock import ScopedClock

B, L, K, D = 2048, 10000, 10, 4
NCORES = 8
BLOC = B // NCORES            # 256 rows per core
NT = BLOC // 128              # 2 partition tiles per core
NCH = 4                       # column chunks per tile
CW = L // NCH                 # 2500
F32 = mybir.dt.float32
I32 = mybir.dt.int32
U32 = mybir.dt.uint32
AX = mybir.AxisListType
OP = mybir.AluOpType
AF = mybir.ActivationFunctionType


def _split_multi_waits(nc):
    """walrus on this image encodes at most ONE sync wait per instruction,
    but Tile's wait assignment attaches several. Move surplus waits onto
    their own same-engine NOPs spliced immediately before the instruction
    (engine streams execute in block order, so semantics are unchanged)."""
    for f in nc.m.functions:
        for blk in f.blocks:
            bb = blk.bb if hasattr(blk, "bb") else blk
            new_list = []
            changed = False
            for inst in bb.instructions:
                si = inst.sync_info
                if si is not None and si.on_wait and len(si.on_wait) > 1:
                    changed = True
                    waits = list(si.on_wait)
                    for w in waits[:-1]:
                        nop = mybir.InstNoOp(
                            name=f"I-ws{nc.next_id()}",
                            engine=inst.engine,
                            sync_info=mybir.SyncInfo(on_wait=[w], on_update=[]),
                        )
                        nc.register_instruction(nop)
                        new_list.append(nop)
                    si.on_wait = [waits[-1]]
                new_list.append(inst)
            if changed:
                bb.instructions[:] = new_list


def build_nc(reps=1, hw_loop=False):
    nc = bass.Bass("TRN2")
    x = nc.dram_tensor("x", [BLOC, L], F32, kind="ExternalInput")
    tgt_blk = nc.dram_tensor("tgt_blk", [NT, 128, 1], I32, kind="ExternalInput")
    rowbasef = nc.dram_tensor("rowbasef", [NT, 128, 1], F32, kind="ExternalInput")
    w1 = nc.dram_tensor("w1", [NT, 128, K], F32, kind="ExternalInput")
    w2 = nc.dram_tensor("w2", [NT, 128, 100], F32, kind="ExternalInput")
    ib100_d = nc.dram_tensor("ib100", [128, 100], F32, kind="ExternalInput")
    win_o = nc.dram_tensor("win", [NT, 128, 1], F32, kind="ExternalOutput")
    den_o = nc.dram_tensor("den", [NT, 128, 1], F32, kind="ExternalOutput")
    preds_o = nc.dram_tensor("preds", [NT, 128, 1], I32, kind="ExternalOutput")

    xg100 = x.ap().rearrange("r (a l) -> (r a) l", l=100)   # [25600, 100]
    xg10 = x.ap().rearrange("r (a l) -> (r a) l", l=10)     # [256000, 10]

    with tile.TileContext(nc) as tc:
        with (
            tc.tile_pool(name="consts", bufs=1) as consts,
            tc.tile_pool(name="xin", bufs=3) as xin,
            tc.tile_pool(name="ebuf", bufs=2) as ebuf,
            tc.tile_pool(name="small", bufs=2) as small,
        ):
            ib = consts.tile([128, 100], F32)
            nc.sync.dma_start(out=ib[:], in_=ib100_d.ap()[:, :])

            def emit_tile(t):
                rows = slice(t * 128, (t + 1) * 128)
                p2 = small.tile([128, 100], F32, tag="p2")
                dsum = small.tile([128, NCH], F32, tag="dsum")
                for c in range(NCH):
                    xc = xin.tile([128, CW], F32, tag="xc")
                    nc.sync.dma_start(out=xc[:], in_=x.ap()[rows, c * CW:(c + 1) * CW])
                    ec = ebuf.tile([128, CW], F32, tag="ec")
                    nc.scalar.activation(ec[:], xc[:], AF.Exp,
                                         accum_out=dsum[:, c:c + 1])
                    nc.vector.tensor_reduce(
                        out=p2[:, c * 25:(c + 1) * 25],
                        in_=ec[:].rearrange("p (a l) -> p a l", l=100),
                        axis=AX.X, op=OP.add)

                den = small.tile([128, 1], F32, tag="den")
                nc.vector.tensor_reduce(out=den[:], in_=dsum[:], axis=AX.X, op=OP.add)
                nc.sync.dma_start(out=den_o.ap()[t], in_=den[:])

                p3 = small.tile([128, K], F32, tag="p3")
                nc.vector.tensor_reduce(
                    out=p3[:], in_=p2[:].rearrange("p (a l) -> p a l", l=10),
                    axis=AX.X, op=OP.add)

                # --- greedy descent: root -> level-3 -> level-2 ---
                mx3 = small.tile([128, 8], F32, tag="mx3")
                nc.vector.max(out=mx3[:], in_=p3[:])
                j3 = small.tile([128, 8], U32, tag="j3")
                nc.vector.max_index(out=j3[:], in_max=mx3[:], in_values=p3[:])
                j3f = small.tile([128, 1], F32, tag="j3f")
                nc.vector.tensor_copy(j3f[:], j3[:, 0:1])

                m2 = small.tile([128, 100], F32, tag="m2")
                nc.vector.scalar_tensor_tensor(
                    out=m2[:], in0=ib[:], scalar=j3f[:, 0:1], in1=p2[:],
                    op0=OP.is_equal, op1=OP.mult)
                mx2 = small.tile([128, 8], F32, tag="mx2")
                nc.vector.max(out=mx2[:], in_=m2[:])
                j2 = small.tile([128, 8], U32, tag="j2")
                nc.vector.max_index(out=j2[:], in_max=mx2[:], in_values=m2[:])
                j2f = small.tile([128, 1], F32, tag="j2f")
                nc.vector.tensor_copy(j2f[:], j2[:, 0:1])

                # gather the 100 leaf logits under level-2 node j2
                rb = small.tile([128, 1], F32, tag="rb")
                nc.sync.dma_start(out=rb[:], in_=rowbasef.ap()[t])
                descf = small.tile([128, 1], F32, tag="descf")
                nc.vector.tensor_add(descf[:], j2f[:], rb[:])
                desci = small.tile([128, 1], I32, tag="desci")
                nc.vector.tensor_copy(desci[:], descf[:])
                g100 = small.tile([128, 100], F32, tag="g100")
                nc.gpsimd.indirect_dma_start(
                    out=g100[:], out_offset=None, in_=xg100,
                    in_offset=bass.IndirectOffsetOnAxis(ap=desci[:, 0:1], axis=0))
                e100 = small.tile([128, 100], F32, tag="e100")
                nc.scalar.activation(e100[:], g100[:], AF.Exp)

                # level-1 step then leaf step, both inside the gathered block
                p1b = small.tile([128, K], F32, tag="p1b")
                nc.vector.tensor_reduce(
                    out=p1b[:], in_=e100[:].rearrange("p (a l) -> p a l", l=10),
                    axis=AX.X, op=OP.add)
                mx1 = small.tile([128, 8], F32, tag="mx1")
                nc.vector.max(out=mx1[:], in_=p1b[:])
                j1 = small.tile([128, 8], U32, tag="j1")
                nc.vector.max_index(out=j1[:], in_max=mx1[:], in_values=p1b[:])
                j1f = small.tile([128, 1], F32, tag="j1f")
                nc.vector.tensor_copy(j1f[:], j1[:, 0:1])

                m0 = small.tile([128, 100], F32, tag="m0")
                nc.vector.scalar_tensor_tensor(
                    out=m0[:], in0=ib[:], scalar=j1f[:, 0:1], in1=e100[:],
                    op0=OP.is_equal, op1=OP.mult)
                mx0 = small.tile([128, 8], F32, tag="mx0")
                nc.vector.max(out=mx0[:], in_=m0[:])
                c0 = small.tile([128, 8], U32, tag="c0")
                nc.vector.max_index(out=c0[:], in_max=mx0[:], in_values=m0[:])
                c0f = small.tile([128, 1], F32, tag="c0f")
                nc.vector.tensor_copy(c0f[:], c0[:, 0:1])

                predf = small.tile([128, 1], F32, tag="predf")
                nc.vector.scalar_tensor_tensor(
                    out=predf[:], in0=j2f[:], scalar=100.0, in1=c0f[:],
                    op0=OP.mult, op1=OP.add)
                predi = small.tile([128, 1], I32, tag="predi")
                nc.vector.tensor_copy(predi[:], predf[:])
                nc.sync.dma_start(out=preds_o.ap()[t], in_=predi[:])

                # --- win_u: path-weighted prob sum for the loss ---
                tb = small.tile([128, 1], I32, tag="tb")
                nc.sync.dma_start(out=tb[:], in_=tgt_blk.ap()[t])
                xb10 = small.tile([128, K], F32, tag="xb10")
                nc.gpsimd.indirect_dma_start(
                    out=xb10[:], out_offset=None, in_=xg10,
                    in_offset=bass.IndirectOffsetOnAxis(ap=tb[:, 0:1], axis=0))
                e10 = small.tile([128, K], F32, tag="e10")
                nc.scalar.activation(e10[:], xb10[:], AF.Exp)

                w1t = small.tile([128, K], F32, tag="w1t")
                nc.sync.dma_start(out=w1t[:], in_=w1.ap()[t])
                w2t = small.tile([128, 100], F32, tag="w2t")
                nc.sync.dma_start(out=w2t[:], in_=w2.ap()[t])

                t10 = small.tile([128, K], F32, tag="t10")
                win1 = small.tile([128, 1], F32, tag="win1")
                nc.vector.scalar_tensor_tensor(
                    out=t10[:], in0=e10[:], scalar=1.0, in1=w1t[:],
                    op0=OP.mult, op1=OP.mult, accum_out=win1[:])
                t100 = small.tile([128, 100], F32, tag="t100")
                win2 = small.tile([128, 1], F32, tag="win2")
                nc.vector.scalar_tensor_tensor(
                    out=t100[:], in0=p2[:], scalar=1.0, in1=w2t[:],
                    op0=OP.mult, op1=OP.mult, accum_out=win2[:])
                winv = small.tile([128, 1], F32, tag="winv")
                nc.vector.tensor_add(winv[:], win1[:], win2[:])
                nc.sync.dma_start(out=win_o.ap()[t], in_=winv[:])

            if hw_loop:
                with tc.For_i(0, reps, 1) as _i:
                    for t in range(NT):
                        emit_tile(t)
            else:
                for _ in range(reps):
                    for t in range(NT):
                        emit_tile(t)
    _split_multi_waits(nc)
    return nc


def make_in_maps(x_full, tgt_full):
    x_full = np.ascontiguousarray(np.asarray(x_full, dtype=np.float32))
    tgt_full = np.asarray(tgt_full).astype(np.int64)
    ib100 = np.ascontiguousarray(
        np.broadcast_to(np.arange(100, dtype=np.int64) // 10, (128, 100))
    ).astype(np.float32)
    rowbasef = (np.arange(BLOC) * 100).astype(np.float32).reshape(NT, 128, 1)
    in_maps = []
    for i in range(NCORES):
        rs = slice(i * BLOC, (i + 1) * BLOC)
        t = tgt_full[rs]
        t1, r = t // 10, t % 10
        t2, t3 = t // 100, t // 1000
        tgt_blk = (np.arange(BLOC) * 1000 + t1).astype(np.int32).reshape(NT, 128, 1)
        w1 = (0.125 * (1.0 + (np.arange(K)[None, :] == r[:, None]))) \
            .astype(np.float32).reshape(NT, 128, K)
        w2 = (0.25 * (np.arange(100)[None, :] == t2[:, None])
              + 0.5 * ((np.arange(100)[None, :] // 10) == t3[:, None])) \
            .astype(np.float32).reshape(NT, 128, 100)
        in_maps.append(dict(
            x=np.ascontiguousarray(x_full[rs]), tgt_blk=tgt_blk,
            rowbasef=rowbasef, w1=w1, w2=w2, ib100=ib100))
    return in_maps


def finalize(results, tgt_full):
    tgt_full = np.asarray(tgt_full).astype(np.int64)
    win = np.concatenate([np.asarray(r["win"]).reshape(-1) for r in results])
    den = np.concatenate([np.asarray(r["den"]).reshape(-1) for r in results])
    preds = np.concatenate(
        [np.asarray(r["preds"]).reshape(-1) for r in results]).astype(np.int32)
    loss = np.float32(np.mean(np.log(den) - np.log(win)))
    pq = preds.astype(np.int64)
    dist = np.full(B, D, dtype=np.int64)
    for m in range(D - 1, -1, -1):
        dist = np.where(pq // 10**m == tgt_full // 10**m, m, dist)
    total_dist = np.asarray([dist.sum()], dtype=np.float32)
    return loss, preds, total_dist


_NC = None


def kernel(outputs, target, chain=None, children=None):
    global _NC
    if _NC is None:
        _NC = build_nc()
    in_maps = make_in_maps(outputs, target)
    res = run_bass_kernel_spmd(_NC, in_maps, core_ids=list(range(NCORES)))
    return finalize(res.results, target)
